# revision 18
# baseline (speedup 1.0000x reference)
"""DKT (Deep Knowledge Tracing) accumulate-concat model on 8 Trainium2 cores.

Model (per example): one-hot interactions x[t] (2S=1024), query one-hots q,
  emb   = x @ W_emb + b_emb
  count = cumulative count state (c_t = sum(x_t)*c_{t-1} + x_t; x one-hot => cumsum)
  z     = [emb, log1p(count), log1p(delta)]
  h     = LSTM(z)                      (Keras gates i,f,g,o; unit forget bias)
  y     = sum(sigmoid(h @ W_out + b_out) * q, -1)

Sharding: data-parallel over batch. 8 cores x 8 examples. Weights replicated.

Device algorithm per core (B'=8 examples):
  Phase 0: cast weights to fp16; W_lstm re-tiled into DRAM as
    [16 m][128 p][10 kc][128 c] so phase 1 reads one contiguous-per-partition
    320KB DMA per m-group (2.5KB bursts).
  Phase 1 (per example):
    countT[2S, T] = x^T cumsum over t as ONE matmul (lhsT = x fp8 one-hot,
      rhs = fp8 upper-triangular ones; exact, fp32 psum).
    zt[2+mc] = log1p(countT) (ACT Ln, bias=1); cnt16 = fp16 copy of countT.
    embT = column-diff(W_emb^T @ cnt16) + b_emb   (diff recovers one-hot xT)
    WzT[2048, T] = W_lstm^T z + bias, fp16 in SBUF wz[p, m, t, b].
  Phase 2 (recurrence): per step, three PSUM tiles (i+f shared, g, o).
    Each tile's accumulation group starts with an identity matmul that loads
    wz[t] into PSUM, then U^T h matmuls accumulate on top. ACT reads PSUM
    directly: one sigmoid over [i|f], tanh(g), tanh(c), sigmoid(o) -- 4 ACT
    ops/step, no DVE psum reads. DVE: cf, ig, cnew, h (SBUF-only, ~60ns).
  Phase 3 (fused, every 16 steps): s = h @ W_out + b_out,
    y = reduce_sum(sigmoid(s) * q); contiguous [16, 8] store.

Output DRAM tensor is [T, 8] (t-major); host transposes.
"""

import os
import sys

sys.path.insert(0, "/opt/trn_rl_repo")

KSTOP = int(os.environ.get("KSTOP", "9"))  # debug: stop after phase N

import numpy as np

import concourse.bass as bass
import concourse.tile as tile
from concourse import bacc, mybir
from concourse.bass_utils import run_bass_kernel_spmd

F32 = mybir.dt.float32
F16 = mybir.dt.float16
F8 = mybir.dt.float8e4
AF = mybir.ActivationFunctionType
ALU = mybir.AluOpType

N_CORES = 8
B_FULL, T_FULL, S = 64, 512, 512
S2 = 2 * S          # 1024 one-hot dim
DE = 256            # emb dim
H = 512             # lstm hidden
G4 = 4 * H          # 2048 gate cols
BP = 8              # examples per core


def _build(T=T_FULL):
    KT = T // 128           # K-tiles over time for count matmul
    nc = bacc.Bacc("TRN2", target_bir_lowering=False, debug=False)

    x_h = nc.dram_tensor("x", [BP, T, S2], F32, kind="ExternalInput")
    d_h = nc.dram_tensor("delta", [BP, T], F32, kind="ExternalInput")
    q_h = nc.dram_tensor("q", [BP, T, S], F32, kind="ExternalInput")
    we_h = nc.dram_tensor("W_emb", [S2, DE], F32, kind="ExternalInput")
    be_h = nc.dram_tensor("b_emb", [DE], F32, kind="ExternalInput")
    wl_h = nc.dram_tensor("W_lstm", [S2 + DE + 1, G4], F32, kind="ExternalInput")
    ul_h = nc.dram_tensor("U_lstm", [H, G4], F32, kind="ExternalInput")
    bl_h = nc.dram_tensor("b_lstm", [G4], F32, kind="ExternalInput")
    wo_h = nc.dram_tensor("W_out", [H, S], F32, kind="ExternalInput")
    bo_h = nc.dram_tensor("b_out", [S], F32, kind="ExternalInput")
    y_h = nc.dram_tensor("y", [T, BP], F32, kind="ExternalOutput")

    # fp16 weight scratch in DRAM; W_lstm tiled [16 m][128 p][10 kc][128 c]
    wlt_h = nc.dram_tensor("wl_t16", [16, 128, 10, 128], F16, kind="Internal")
    ulh_h = nc.dram_tensor("ul_f16", [H, G4], F16, kind="Internal")
    # staged h for the phase-3 epilogue: [4 ko][128 p][T][BP]
    hs_h = nc.dram_tensor("h_stage", [4, 128, T, BP], F16, kind="Internal")

    import ml_dtypes
    tri_h = nc.inline_tensor(
        np.triu(np.ones((T, T))).astype(ml_dtypes.float8_e4m3), name="triu")
    eye_h = nc.inline_tensor(np.eye(128, dtype=np.float16), name="eye128")

    x, d, q = x_h.ap(), d_h.ap(), q_h.ap()
    we, be, wl, ul, bl = we_h.ap(), be_h.ap(), wl_h.ap(), ul_h.ap(), bl_h.ap()
    wo, bo, y = wo_h.ap(), bo_h.ap(), y_h.ap()
    wlt, ulh, tri, eye = wlt_h.ap(), ulh_h.ap(), tri_h.ap(), eye_h.ap()
    hs = hs_h.ap()

    with tile.TileContext(nc) as tc:
        _kernel_body(nc, tc, T, KT, x, d, q, we, be, wl, ul, bl, wo, bo, y,
                     wlt, ulh, tri, eye, hs)
    nc.compile()
    return nc


def _kernel_body(nc, tc, T, KT, x, d, q, we, be, wl, ul, bl, wo, bo, y,
                 wlt, ulh, tri, eye, hs):
    from contextlib import ExitStack

    ctx = ExitStack()
    with ctx:
        # ---------- persistent pools ----------
        per = ctx.enter_context(tc.tile_pool(name="persist", bufs=1))
        wz_pool = ctx.enter_context(tc.tile_pool(name="wz", bufs=1))

        wz = wz_pool.tile([128, 16, T, BP], F16)          # 128KB/part at T=512
        wo_sb = per.tile([128, 4, S], F16)                # 4KB/part
        wemb = per.tile([128, 8, DE], F16)                # 4KB/part
        tri_sb = per.tile([128, KT, T], F8)               # 2KB/part at T=512
        eye_sb = per.tile([128, 128], F16)
        wd = per.tile([1, G4], F16)
        bembT = per.tile([128, 2], F32)
        bembT16 = per.tile([128, 2], F16)
        blstm = per.tile([128, 16], F32)
        bias_g = per.tile([128, 16], F32)                 # b_lstm + b_emb@W1
        bout16 = per.tile([1, S], F16)
        ones1 = per.tile([1, 128], F16)
        nc.vector.memset(ones1, 1.0)
        nc.sync.dma_start(out=eye_sb, in_=eye)

        # ---------- phase 0: load + cast weights ----------
        with tc.tile_pool(name="ph0", bufs=1) as p0, \
             tc.tile_pool(name="ph0ps", bufs=1, space="PSUM") as p0ps:
            # U -> fp16 DRAM scratch (loaded to SBUF at phase 2 start)
            for ko in range(4):
                t32 = p0.tile([128, G4], F32, tag="w32")
                t16 = p0.tile([128, G4], F16, tag="w16")
                nc.sync.dma_start(out=t32, in_=ul[128 * ko:128 * (ko + 1), :])
                nc.scalar.activation(t16, t32, AF.Copy)
                nc.sync.dma_start(out=ulh[128 * ko:128 * (ko + 1), :], in_=t16)
            for ko in range(4):
                t32 = p0.tile([128, S], F32, tag="wo32")
                nc.sync.dma_start(out=t32, in_=wo[128 * ko:128 * (ko + 1), :])
                nc.scalar.activation(wo_sb[:, ko, :], t32, AF.Copy)
            for mc in range(8):
                t32 = p0.tile([128, DE], F32, tag="we32")
                nc.sync.dma_start(out=t32, in_=we[128 * mc:128 * (mc + 1), :])
                nc.scalar.activation(wemb[:, mc, :], t32, AF.Copy)
            for kt in range(KT):
                nc.sync.dma_start(out=tri_sb[:, kt, :],
                                  in_=tri[128 * kt:128 * (kt + 1), :])
            # W_lstm rows 0..1279 -> tiled fp16 DRAM scratch:
            # element (m, p, kc, c) at m*163840 + p*1280 + kc*128 + c
            for kc in range(10):
                t32 = p0.tile([128, G4], F32, tag="w32")
                t16 = p0.tile([128, G4], F16, tag="w16")
                nc.sync.dma_start(out=t32, in_=wl[128 * kc:128 * (kc + 1), :])
                nc.scalar.activation(t16, t32, AF.Copy)
                nc.sync.dma_start(
                    out=bass.AP(tensor=wlt.tensor,
                                offset=wlt.offset + kc * 128,
                                ap=[[1280, 128], [163840, 16], [1, 128]]),
                    in_=t16)
            # delta row of W_lstm
            t32 = p0.tile([1, G4], F32, tag="wd32")
            nc.sync.dma_start(out=t32, in_=wl[1280:1281, :])
            nc.scalar.activation(wd, t32, AF.Copy)
            # biases
            t32 = p0.tile([128, 2], F32, tag="be32")
            nc.sync.dma_start(
                out=t32,
                in_=bass.AP(tensor=be.tensor, offset=be.offset,
                            ap=[[1, 128], [128, 2]]),
            )
            nc.vector.tensor_copy(bembT, t32)
            nc.vector.tensor_copy(bembT16, t32)
            nc.sync.dma_start(
                out=blstm,
                in_=bass.AP(tensor=bl.tensor, offset=bl.offset,
                            ap=[[1, 128], [128, 16]]),
            )
            t32 = p0.tile([1, S], F32, tag="bo32")
            nc.sync.dma_start(out=t32, in_=bo[None, :])
            nc.scalar.activation(bout16, t32, AF.Copy)
            # bias_g[:, m] = b_lstm[m] + (b_emb @ W1)[m]
            for m in range(16):
                bias_ps = p0ps.tile([128, 1], F32, tag="biasps")
                for kc in range(2):
                    wt16 = p0.tile([128, 128], F16, tag="wbt")
                    nc.sync.dma_start(
                        out=wt16,
                        in_=bass.AP(tensor=wlt.tensor,
                                    offset=wlt.offset + m * 163840 + kc * 128,
                                    ap=[[1280, 128], [1, 128]]))
                    nc.tensor.matmul(bias_ps, wt16, bembT16[:, kc:kc + 1],
                                     start=(kc == 0), stop=(kc == 1))
                nc.vector.tensor_add(bias_g[:, m:m + 1], bias_ps,
                                     blstm[:, m:m + 1])

        if KSTOP < 1:
            return
        # ---------- phase 1: Wz precompute ----------
        with tc.tile_pool(name="p1sb", bufs=1) as p1, \
             tc.tile_pool(name="p1s", bufs=1) as p1s, \
             tc.tile_pool(name="p1x", bufs=2) as p1x, \
             tc.tile_pool(name="p1w", bufs=3) as p1w, \
             tc.tile_pool(name="p1cnt", bufs=1, space="PSUM") as pps, \
             tc.tile_pool(name="p1mm", bufs=2, space="PSUM") as pps2:
            zt = {}
            for kc in range(10):
                zt[kc] = p1.tile([128, T], F16, tag=f"zt{kc}", name=f"zt{kc}")
            for ex in range(BP):
                ld32 = p1x.tile([1, T], F32, tag="ld32")
                nc.sync.dma_start(out=ld32, in_=d[ex:ex + 1, :])
                ldt = p1x.tile([1, T], F16, tag="ldt")
                nc.scalar.activation(ldt, ld32, AF.Ln, bias=1.0)
                # -- load + cast x once (fp8: one-hot, exact) --
                x8 = p1s.tile([128, KT, S2], F8, tag="x8")
                for kt in range(KT):
                    x32 = p1x.tile([128, S2], F32, tag="x32")
                    nc.sync.dma_start(
                        out=x32, in_=x[ex, 128 * kt:128 * (kt + 1), :])
                    nc.scalar.activation(x8[:, kt, :], x32, AF.Copy)
                # -- count matmul + log1p + fp16 copy --
                cnt16 = p1s.tile([128, 8, T], F16, tag="cnt16")
                for mch in range(2):
                    cnt_ps = [pps.tile([128, T], F32, tag=f"cnt{i}",
                                       name=f"cnt{i}")
                              for i in range(4)]
                    for kt in range(KT):
                        for i in range(4):
                            mc = 4 * mch + i
                            nc.tensor.matmul(
                                cnt_ps[i], x8[:, kt, 128 * mc:128 * (mc + 1)],
                                tri_sb[:, kt, :],
                                start=(kt == 0), stop=(kt == KT - 1))
                    for i in range(4):
                        mc = 4 * mch + i
                        nc.scalar.activation(zt[2 + mc], cnt_ps[i], AF.Ln,
                                             bias=1.0)
                        nc.vector.tensor_copy(cnt16[:, mc, :], cnt_ps[i])
                # -- embT = diff(W_emb^T @ cnt16) + b_emb --
                for m2 in range(2):
                    e_ps = pps2.tile([128, T], F32, tag="mm")
                    for mc in range(8):
                        nc.tensor.matmul(
                            e_ps, wemb[:, mc, 128 * m2:128 * (m2 + 1)],
                            cnt16[:, mc, :],
                            start=(mc == 0), stop=(mc == 7))
                    e_sb = p1x.tile([128, T], F32, tag="esb")
                    nc.vector.tensor_copy(e_sb, e_ps)
                    nc.vector.tensor_scalar_add(zt[m2][:, 0:1], e_sb[:, 0:1],
                                                bembT[:, m2:m2 + 1])
                    nc.vector.tensor_sub(zt[m2][:, 1:T], e_sb[:, 1:T],
                                         e_sb[:, 0:T - 1])
                    nc.vector.tensor_scalar_add(zt[m2][:, 1:T], zt[m2][:, 1:T],
                                                bembT[:, m2:m2 + 1])
                # -- big matmul: WzT = W^T z + bias --
                for m in range(16):
                    wtile = p1w.tile([128, 10, 128], F16, tag="wlt")
                    nc.sync.dma_start(
                        out=wtile,
                        in_=bass.AP(tensor=wlt.tensor,
                                    offset=wlt.offset + m * 163840,
                                    ap=[[1280, 128], [128, 10], [1, 128]]))
                    b_ps = pps2.tile([128, T], F32, tag="mm")
                    for kc in range(10):
                        nc.tensor.matmul(b_ps, wtile[:, kc, :], zt[kc],
                                         start=(kc == 0), stop=False)
                    nc.tensor.matmul(b_ps, wd[:, 128 * m:128 * (m + 1)],
                                     ldt,
                                     start=False, stop=True)
                    nc.vector.tensor_scalar_add(wz[:, m, :, ex], b_ps,
                                                bias_g[:, m:m + 1])

        if KSTOP < 2:
            return
        # ---------- phase 2 + 3: recurrence with fused output ----------
        with tc.tile_pool(name="rec", bufs=1) as rp, \
             tc.tile_pool(name="recd", bufs=2) as rd, \
             tc.tile_pool(name="act4", bufs=2) as ap4, \
             tc.tile_pool(name="pif", bufs=2, space="PSUM") as pif, \
             tc.tile_pool(name="pg", bufs=2, space="PSUM") as pg, \
             tc.tile_pool(name="po", bufs=2, space="PSUM") as po, \
             tc.tile_pool(name="sps", bufs=2, space="PSUM") as sps:
            # U fp16 from DRAM scratch (SBUF space freed by phase-1 pools)
            u_sb = rp.tile([128, 4, G4], F16)             # 16KB/part
            for ko in range(4):
                nc.sync.dma_start(out=u_sb[:, ko, :],
                                  in_=ulh[128 * ko:128 * (ko + 1), :])
            hring = rp.tile([128, 4, 33, BP], F16)
            c0 = rp.tile([128, 4, BP], F32)
            nc.vector.memset(hring[:, :, 0, :], 0.0)
            nc.vector.memset(c0, 0.0)
            cprev = c0

            for t in range(T):
                sl_prev = 1 + ((t - 1) % 32) if t > 0 else 0
                sl = 1 + (t % 32)
                hprev = [hring[:, ko, sl_prev, :] for ko in range(4)]
                # PSUM tiles; each accumulation group starts with an identity
                # matmul that loads wz[t], then U^T h accumulates on top.
                # Matmuls are ko-major: all ko=0,1 first (they only need the
                # first half of h(t-1), written early by the split tail below).
                if_ps = pif.tile([128, 8, BP], F32, tag="if")
                g_ps = pg.tile([128, 4, BP], F32, tag="g")
                o_ps = po.tile([128, 4, BP], F32, tag="o")
                nc.tensor.matmul(if_ps, eye_sb, wz[:, 0:8, t, :],
                                 start=True, stop=False, skip_group_check=True)
                nc.tensor.matmul(g_ps, eye_sb, wz[:, 8:12, t, :],
                                 start=True, stop=False, skip_group_check=True)
                nc.tensor.matmul(o_ps, eye_sb, wz[:, 12:16, t, :],
                                 start=True, stop=False, skip_group_check=True)
                tiles = ([(if_ps, jo, jo) for jo in range(8)]
                         + [(g_ps, jo, 8 + jo) for jo in range(4)]
                         + [(o_ps, jo, 12 + jo) for jo in range(4)])
                for kh in range(2):
                    for ko in (2 * kh, 2 * kh + 1):
                        for ps, jo, m in tiles:
                            nc.tensor.matmul(
                                ps[:, jo, :],
                                u_sb[:, ko, 128 * m:128 * (m + 1)],
                                hprev[ko],
                                start=False,
                                stop=(ko == 3 and m in (7, 11, 15)),
                                skip_group_check=True)
                s_if = ap4.tile([128, 8, BP], F32, tag="sif")
                nc.scalar.activation(s_if, if_ps, AF.Sigmoid)
                tg = ap4.tile([128, 4, BP], F32, tag="tg")
                nc.scalar.activation(tg, g_ps, AF.Tanh)
                cf = ap4.tile([128, 4, BP], F32, tag="cf")
                nc.vector.tensor_mul(cf, s_if[:, 4:8, :], cprev)
                ig = ap4.tile([128, 4, BP], F32, tag="ig")
                nc.vector.tensor_mul(ig, s_if[:, 0:4, :], tg)
                cnew = ap4.tile([128, 4, BP], F32, tag="c")
                nc.vector.tensor_add(cnew, cf, ig)
                so = ap4.tile([128, 4, BP], F32, tag="so")
                th = ap4.tile([128, 4, BP], F32, tag="th")
                # tail in jo-halves: h[0:2] lands first, releasing next
                # step's ko=0,1 matmuls while the second half completes
                for hf in range(2):
                    j0, j1 = 2 * hf, 2 * hf + 2
                    nc.scalar.activation(th[:, j0:j1, :], cnew[:, j0:j1, :],
                                         AF.Tanh)
                    nc.scalar.activation(so[:, j0:j1, :], o_ps[:, j0:j1, :],
                                         AF.Sigmoid)
                    nc.vector.tensor_mul(hring[:, j0:j1, sl, :],
                                         so[:, j0:j1, :], th[:, j0:j1, :])
                cprev = cnew

                if t % 16 == 15 and KSTOP >= 3:
                    # stage h to DRAM for the phase-3 epilogue
                    t0 = t - 15
                    sl0 = 1 + (t0 % 32)
                    for ko in range(4):
                        nc.sync.dma_start(
                            out=bass.AP(tensor=hs.tensor,
                                        offset=hs.offset
                                        + (ko * 128 * T + t0) * BP,
                                        ap=[[T * BP, 128], [BP, 16], [1, BP]]),
                            in_=hring[:, ko, sl0:sl0 + 16, :])

            # ---------- phase 3 epilogue ----------
            if KSTOP >= 3:
                for blk in range(T // 16):
                    t0 = 16 * blk
                    hb = rd.tile([128, 4, 16, BP], F16, tag="hb")
                    for ko in range(4):
                        nc.sync.dma_start(
                            out=hb[:, ko, :, :],
                            in_=bass.AP(tensor=hs.tensor,
                                        offset=hs.offset
                                        + (ko * 128 * T + t0) * BP,
                                        ap=[[T * BP, 128], [BP, 16], [1, BP]]))
                    s_ps = sps.tile([128, S], F32, tag="sps")
                    for ko in range(4):
                        nc.tensor.matmul(
                            s_ps, hb[:, ko, :, :], wo_sb[:, ko, :],
                            start=(ko == 0), stop=False)
                    nc.tensor.matmul(s_ps, ones1, bout16,
                                     start=False, stop=True)
                    sig = rd.tile([128, S], F32, tag="sig")
                    nc.scalar.activation(sig, s_ps, AF.Sigmoid)
                    q_t = rd.tile([128, S], F32, tag="qt")
                    nc.sync.dma_start(
                        out=q_t,
                        in_=bass.AP(tensor=q.tensor,
                                    offset=q.offset + t0 * S,
                                    ap=[[S, 16], [T * S, BP], [1, S]]),
                    )
                    prod = rd.tile([128, S], F32, tag="prod")
                    ycol = rd.tile([128, 1], F32, tag="ycol")
                    nc.vector.tensor_mul(prod, sig, q_t)
                    nc.vector.tensor_reduce(ycol, prod, mybir.AxisListType.X,
                                            ALU.add)
                    nc.sync.dma_start(out=y[t0:t0 + 16, :], in_=ycol)


_CACHE = {}


def _get_nc(T=T_FULL):
    if T not in _CACHE:
        _CACHE[T] = _build(T)
    return _CACHE[T]


def kernel(x, delta, q, W_emb, b_emb, W_lstm, U_lstm, b_lstm, W_out, b_out):
    T = x.shape[1]
    nc = _get_nc(T)
    shared = dict(
        W_emb=np.ascontiguousarray(W_emb, np.float32),
        b_emb=np.ascontiguousarray(b_emb, np.float32),
        W_lstm=np.ascontiguousarray(W_lstm, np.float32),
        U_lstm=np.ascontiguousarray(U_lstm, np.float32),
        b_lstm=np.ascontiguousarray(b_lstm, np.float32),
        W_out=np.ascontiguousarray(W_out, np.float32),
        b_out=np.ascontiguousarray(b_out, np.float32),
    )
    in_maps = []
    for c in range(N_CORES):
        sl = slice(BP * c, BP * (c + 1))
        in_maps.append(dict(
            x=np.ascontiguousarray(x[sl], np.float32),
            delta=np.ascontiguousarray(np.asarray(delta)[sl, :, 0], np.float32),
            q=np.ascontiguousarray(q[sl], np.float32),
            **shared,
        ))
    res = run_bass_kernel_spmd(nc, in_maps, core_ids=list(range(N_CORES)))
    out = np.empty((x.shape[0], T, 1), np.float32)
    for c in range(N_CORES):
        out[BP * c:BP * (c + 1), :, 0] = res.results[c]["y"].T
    return out


# revision 24
# speedup vs baseline: 1.0599x; 1.0599x over previous
"""DKT (Deep Knowledge Tracing) accumulate-concat model on 8 Trainium2 cores.

Model (per example): one-hot interactions x[t] (2S=1024), query one-hots q,
  emb   = x @ W_emb + b_emb
  count = cumulative count state (c_t = sum(x_t)*c_{t-1} + x_t; x one-hot => cumsum)
  z     = [emb, log1p(count), log1p(delta)]
  h     = LSTM(z)                      (Keras gates i,f,g,o; unit forget bias)
  y     = sum(sigmoid(h @ W_out + b_out) * q, -1)

Sharding: data-parallel over batch. 8 cores x 8 examples. Weights replicated.

Device algorithm per core (B'=8 examples):
  Phase 0: cast weights to fp16; W_lstm re-tiled into DRAM as
    [16 m][128 p][10 kc][128 c] so phase 1 reads one contiguous-per-partition
    320KB DMA per m-group (2.5KB bursts).
  Phase 1 (per example):
    countT[2S, T] = x^T cumsum over t as ONE matmul (lhsT = x fp8 one-hot,
      rhs = fp8 upper-triangular ones; exact, fp32 psum).
    zt[2+mc] = log1p(countT) (ACT Ln, bias=1); cnt16 = fp16 copy of countT.
    embT = column-diff(W_emb^T @ cnt16) + b_emb   (diff recovers one-hot xT)
    WzT[2048, T] = W_lstm^T z + bias, fp16 in SBUF wz[p, m, t, b].
  Phase 2 (recurrence): per step, three PSUM tiles (i+f shared, g, o).
    Each tile's accumulation group starts with an identity matmul that loads
    wz[t] into PSUM, then U^T h matmuls accumulate on top. ACT reads PSUM
    directly: one sigmoid over [i|f], tanh(g), tanh(c), sigmoid(o) -- 4 ACT
    ops/step, no DVE psum reads. DVE: cf, ig, cnew, h (SBUF-only, ~60ns).
  Phase 3 (fused, every 16 steps): s = h @ W_out + b_out,
    y = reduce_sum(sigmoid(s) * q); contiguous [16, 8] store.

Output DRAM tensor is [T, 8] (t-major); host transposes.
"""

import os
import sys

sys.path.insert(0, "/opt/trn_rl_repo")

KSTOP = int(os.environ.get("KSTOP", "9"))  # debug: stop after phase N

import numpy as np

import concourse.bass as bass
import concourse.tile as tile
from concourse import bacc, mybir
from concourse.bass_utils import run_bass_kernel_spmd

F32 = mybir.dt.float32
F16 = mybir.dt.float16
F8 = mybir.dt.float8e4
AF = mybir.ActivationFunctionType
ALU = mybir.AluOpType

N_CORES = 8
B_FULL, T_FULL, S = 64, 512, 512
S2 = 2 * S          # 1024 one-hot dim
DE = 256            # emb dim
H = 512             # lstm hidden
G4 = 4 * H          # 2048 gate cols
BP = 8              # examples per core


def _build(T=T_FULL):
    KT = T // 128           # K-tiles over time for count matmul
    nc = bacc.Bacc("TRN2", target_bir_lowering=False, debug=False)

    x_h = nc.dram_tensor("x", [BP, T, S2], F32, kind="ExternalInput")
    d_h = nc.dram_tensor("delta", [BP, T], F32, kind="ExternalInput")
    q_h = nc.dram_tensor("q", [BP, T, S], F32, kind="ExternalInput")
    we_h = nc.dram_tensor("W_emb", [S2, DE], F32, kind="ExternalInput")
    be_h = nc.dram_tensor("b_emb", [DE], F32, kind="ExternalInput")
    wl_h = nc.dram_tensor("W_lstm", [S2 + DE + 1, G4], F32, kind="ExternalInput")
    ul_h = nc.dram_tensor("U_lstm", [H, G4], F32, kind="ExternalInput")
    bl_h = nc.dram_tensor("b_lstm", [G4], F32, kind="ExternalInput")
    wo_h = nc.dram_tensor("W_out", [H, S], F32, kind="ExternalInput")
    bo_h = nc.dram_tensor("b_out", [S], F32, kind="ExternalInput")
    y_h = nc.dram_tensor("y", [T, BP], F32, kind="ExternalOutput")

    # fp16 weight scratch in DRAM; W_lstm tiled [16 m][128 p][10 kc][128 c]
    wlt_h = nc.dram_tensor("wl_t16", [16, 128, 10, 128], F16, kind="Internal")
    ulh_h = nc.dram_tensor("ul_f16", [H, G4], F16, kind="Internal")
    # staged h for the phase-3 epilogue: [4 ko][128 p][T][BP]
    hs_h = nc.dram_tensor("h_stage", [4, 128, T, BP], F16, kind="Internal")

    import ml_dtypes
    tri_h = nc.inline_tensor(
        np.triu(np.ones((T, T))).astype(ml_dtypes.float8_e4m3), name="triu")
    eye_h = nc.inline_tensor(np.eye(128, dtype=np.float16), name="eye128")

    x, d, q = x_h.ap(), d_h.ap(), q_h.ap()
    we, be, wl, ul, bl = we_h.ap(), be_h.ap(), wl_h.ap(), ul_h.ap(), bl_h.ap()
    wo, bo, y = wo_h.ap(), bo_h.ap(), y_h.ap()
    wlt, ulh, tri, eye = wlt_h.ap(), ulh_h.ap(), tri_h.ap(), eye_h.ap()
    hs = hs_h.ap()

    with tile.TileContext(nc) as tc:
        _kernel_body(nc, tc, T, KT, x, d, q, we, be, wl, ul, bl, wo, bo, y,
                     wlt, ulh, tri, eye, hs)
    nc.compile()
    return nc


def _kernel_body(nc, tc, T, KT, x, d, q, we, be, wl, ul, bl, wo, bo, y,
                 wlt, ulh, tri, eye, hs):
    from contextlib import ExitStack

    ctx = ExitStack()
    with ctx:
        # ---------- persistent pools ----------
        per = ctx.enter_context(tc.tile_pool(name="persist", bufs=1))
        wz_pool = ctx.enter_context(tc.tile_pool(name="wz", bufs=1))

        wz = wz_pool.tile([128, 16, T, BP], F16)          # 128KB/part at T=512
        wo_sb = per.tile([128, 4, S], F16)                # 4KB/part
        wemb = per.tile([128, 8, DE], F16)                # 4KB/part
        tri_sb = per.tile([128, KT, T], F8)               # 2KB/part at T=512
        eye_sb = per.tile([128, 128], F16)
        wd = per.tile([1, G4], F16)
        bembT = per.tile([128, 2], F32)
        bembT16 = per.tile([128, 2], F16)
        blstm = per.tile([128, 16], F32)
        bias_g = per.tile([128, 16], F32)                 # b_lstm + b_emb@W1
        bout16 = per.tile([1, S], F16)
        ones1 = per.tile([1, 128], F16)
        nc.vector.memset(ones1, 1.0)
        nc.sync.dma_start(out=eye_sb, in_=eye)

        # ---------- phase 0: load + cast weights ----------
        with tc.tile_pool(name="ph0", bufs=1) as p0, \
             tc.tile_pool(name="ph0ps", bufs=1, space="PSUM") as p0ps:
            # U -> fp16 DRAM scratch (loaded to SBUF at phase 2 start)
            for ko in range(4):
                t32 = p0.tile([128, G4], F32, tag="w32")
                t16 = p0.tile([128, G4], F16, tag="w16")
                nc.sync.dma_start(out=t32, in_=ul[128 * ko:128 * (ko + 1), :])
                nc.scalar.activation(t16, t32, AF.Copy)
                nc.sync.dma_start(out=ulh[128 * ko:128 * (ko + 1), :], in_=t16)
            for ko in range(4):
                t32 = p0.tile([128, S], F32, tag="wo32")
                nc.sync.dma_start(out=t32, in_=wo[128 * ko:128 * (ko + 1), :])
                nc.scalar.activation(wo_sb[:, ko, :], t32, AF.Copy)
            for mc in range(8):
                t32 = p0.tile([128, DE], F32, tag="we32")
                nc.sync.dma_start(out=t32, in_=we[128 * mc:128 * (mc + 1), :])
                nc.scalar.activation(wemb[:, mc, :], t32, AF.Copy)
            for kt in range(KT):
                nc.sync.dma_start(out=tri_sb[:, kt, :],
                                  in_=tri[128 * kt:128 * (kt + 1), :])
            # W_lstm rows 0..1279 -> tiled fp16 DRAM scratch:
            # element (m, p, kc, c) at m*163840 + p*1280 + kc*128 + c
            for kc in range(10):
                t32 = p0.tile([128, G4], F32, tag="w32")
                t16 = p0.tile([128, G4], F16, tag="w16")
                nc.sync.dma_start(out=t32, in_=wl[128 * kc:128 * (kc + 1), :])
                nc.scalar.activation(t16, t32, AF.Copy)
                nc.sync.dma_start(
                    out=bass.AP(tensor=wlt.tensor,
                                offset=wlt.offset + kc * 128,
                                ap=[[1280, 128], [163840, 16], [1, 128]]),
                    in_=t16)
            # delta row of W_lstm
            t32 = p0.tile([1, G4], F32, tag="wd32")
            nc.sync.dma_start(out=t32, in_=wl[1280:1281, :])
            nc.scalar.activation(wd, t32, AF.Copy)
            # biases
            t32 = p0.tile([128, 2], F32, tag="be32")
            nc.sync.dma_start(
                out=t32,
                in_=bass.AP(tensor=be.tensor, offset=be.offset,
                            ap=[[1, 128], [128, 2]]),
            )
            nc.vector.tensor_copy(bembT, t32)
            nc.vector.tensor_copy(bembT16, t32)
            nc.sync.dma_start(
                out=blstm,
                in_=bass.AP(tensor=bl.tensor, offset=bl.offset,
                            ap=[[1, 128], [128, 16]]),
            )
            t32 = p0.tile([1, S], F32, tag="bo32")
            nc.sync.dma_start(out=t32, in_=bo[None, :])
            nc.scalar.activation(bout16, t32, AF.Copy)
            # bias_g[:, m] = b_lstm[m] + (b_emb @ W1)[m]
            for m in range(16):
                bias_ps = p0ps.tile([128, 1], F32, tag="biasps")
                for kc in range(2):
                    wt16 = p0.tile([128, 128], F16, tag="wbt")
                    nc.sync.dma_start(
                        out=wt16,
                        in_=bass.AP(tensor=wlt.tensor,
                                    offset=wlt.offset + m * 163840 + kc * 128,
                                    ap=[[1280, 128], [1, 128]]))
                    nc.tensor.matmul(bias_ps, wt16, bembT16[:, kc:kc + 1],
                                     start=(kc == 0), stop=(kc == 1))
                nc.vector.tensor_add(bias_g[:, m:m + 1], bias_ps,
                                     blstm[:, m:m + 1])

        if KSTOP < 1:
            return
        # ---------- phase 1: Wz precompute ----------
        with tc.tile_pool(name="p1sb", bufs=1) as p1, \
             tc.tile_pool(name="p1s", bufs=1) as p1s, \
             tc.tile_pool(name="p1x", bufs=2) as p1x, \
             tc.tile_pool(name="p1w", bufs=3) as p1w, \
             tc.tile_pool(name="p1cnt", bufs=1, space="PSUM") as pps, \
             tc.tile_pool(name="p1mm", bufs=3, space="PSUM") as pps2:
            zt = {}
            for kc in range(10):
                zt[kc] = p1.tile([128, T], F16, tag=f"zt{kc}", name=f"zt{kc}")
            for ex in range(BP):
                ld32 = p1x.tile([1, T], F32, tag="ld32")
                nc.sync.dma_start(out=ld32, in_=d[ex:ex + 1, :])
                ldt = p1x.tile([1, T], F16, tag="ldt")
                nc.scalar.activation(ldt, ld32, AF.Ln, bias=1.0)
                # -- load + cast x once (fp8: one-hot, exact) --
                x8 = p1s.tile([128, KT, S2], F8, tag="x8")
                for kt in range(KT):
                    x32 = p1x.tile([128, S2], F32, tag="x32")
                    nc.sync.dma_start(
                        out=x32, in_=x[ex, 128 * kt:128 * (kt + 1), :])
                    nc.scalar.activation(x8[:, kt, :], x32, AF.Copy)
                # -- count matmul + log1p + fp16 copy --
                cnt16 = p1s.tile([128, 8, T], F16, tag="cnt16")
                for mch in range(2):
                    cnt_ps = [pps.tile([128, T], F32, tag=f"cnt{i}",
                                       name=f"cnt{i}")
                              for i in range(4)]
                    for kt in range(KT):
                        for i in range(4):
                            mc = 4 * mch + i
                            nc.tensor.matmul(
                                cnt_ps[i], x8[:, kt, 128 * mc:128 * (mc + 1)],
                                tri_sb[:, kt, :],
                                start=(kt == 0), stop=(kt == KT - 1))
                    for i in range(4):
                        mc = 4 * mch + i
                        nc.scalar.activation(zt[2 + mc], cnt_ps[i], AF.Ln,
                                             bias=1.0)
                        nc.vector.tensor_copy(cnt16[:, mc, :], cnt_ps[i])
                # -- embT = diff(W_emb^T @ cnt16) + b_emb --
                for m2 in range(2):
                    e_ps = pps2.tile([128, T], F32, tag="mm")
                    for mc in range(8):
                        nc.tensor.matmul(
                            e_ps, wemb[:, mc, 128 * m2:128 * (m2 + 1)],
                            cnt16[:, mc, :],
                            start=(mc == 0), stop=(mc == 7))
                    e_sb = p1x.tile([128, T], F32, tag="esb")
                    nc.vector.tensor_copy(e_sb, e_ps)
                    nc.vector.tensor_scalar_add(zt[m2][:, 0:1], e_sb[:, 0:1],
                                                bembT[:, m2:m2 + 1])
                    nc.vector.tensor_sub(zt[m2][:, 1:T], e_sb[:, 1:T],
                                         e_sb[:, 0:T - 1])
                    nc.vector.tensor_scalar_add(zt[m2][:, 1:T], zt[m2][:, 1:T],
                                                bembT[:, m2:m2 + 1])
                # -- big matmul: WzT = W^T z + bias --
                for m in range(16):
                    wtile = p1w.tile([128, 10, 128], F16, tag="wlt")
                    nc.sync.dma_start(
                        out=wtile,
                        in_=bass.AP(tensor=wlt.tensor,
                                    offset=wlt.offset + m * 163840,
                                    ap=[[1280, 128], [128, 10], [1, 128]]))
                    b_ps = pps2.tile([128, T], F32, tag="mm")
                    for kc in range(10):
                        nc.tensor.matmul(b_ps, wtile[:, kc, :], zt[kc],
                                         start=(kc == 0), stop=False)
                    nc.tensor.matmul(b_ps, wd[:, 128 * m:128 * (m + 1)],
                                     ldt,
                                     start=False, stop=True)
                    nc.vector.tensor_scalar_add(wz[:, m, :, ex], b_ps,
                                                bias_g[:, m:m + 1])

        if KSTOP < 2:
            return
        # ---------- phase 2 + 3: recurrence with fused output ----------
        with tc.tile_pool(name="rec", bufs=1) as rp, \
             tc.tile_pool(name="recd", bufs=2) as rd, \
             tc.tile_pool(name="act4", bufs=2) as ap4, \
             tc.tile_pool(name="pif", bufs=2, space="PSUM") as pif, \
             tc.tile_pool(name="pg", bufs=2, space="PSUM") as pg, \
             tc.tile_pool(name="po", bufs=2, space="PSUM") as po, \
             tc.tile_pool(name="sps", bufs=2, space="PSUM") as sps:
            # U fp16 from DRAM scratch (SBUF space freed by phase-1 pools)
            u_sb = rp.tile([128, 4, G4], F16)             # 16KB/part
            for ko in range(4):
                nc.sync.dma_start(out=u_sb[:, ko, :],
                                  in_=ulh[128 * ko:128 * (ko + 1), :])
            hring = rp.tile([128, 4, 33, BP], F16)
            c0 = rp.tile([128, 4, BP], F32)
            nc.vector.memset(hring[:, :, 0, :], 0.0)
            nc.vector.memset(c0, 0.0)
            cprev = c0

            for t in range(T):
                sl_prev = 1 + ((t - 1) % 32) if t > 0 else 0
                sl = 1 + (t % 32)
                hprev = [hring[:, ko, sl_prev, :] for ko in range(4)]
                # ---- if-tile: identity loads wz, then U^T h accumulates ----
                if_ps = pif.tile([128, 8, BP], F32, tag="if")
                nc.tensor.matmul(if_ps, eye_sb, wz[:, 0:8, t, :],
                                 start=True, stop=False, skip_group_check=True)
                for jo in range(8):
                    for ko in range(4):
                        nc.tensor.matmul(
                            if_ps[:, jo, :],
                            u_sb[:, ko, 128 * jo:128 * (jo + 1)],
                            hprev[ko],
                            start=False, stop=(jo == 7 and ko == 3),
                            skip_group_check=True)
                s_if = ap4.tile([128, 8, BP], F32, tag="sif")
                nc.scalar.activation(s_if, if_ps, AF.Sigmoid)
                cf = ap4.tile([128, 4, BP], F32, tag="cf")
                nc.vector.tensor_mul(cf, s_if[:, 4:8, :], cprev)
                # ---- g-tile ----
                g_ps = pg.tile([128, 4, BP], F32, tag="g")
                nc.tensor.matmul(g_ps, eye_sb, wz[:, 8:12, t, :],
                                 start=True, stop=False, skip_group_check=True)
                for jo in range(4):
                    m = 8 + jo
                    for ko in range(4):
                        nc.tensor.matmul(
                            g_ps[:, jo, :],
                            u_sb[:, ko, 128 * m:128 * (m + 1)],
                            hprev[ko],
                            start=False, stop=(jo == 3 and ko == 3),
                            skip_group_check=True)
                tg = ap4.tile([128, 4, BP], F32, tag="tg")
                nc.scalar.activation(tg, g_ps, AF.Tanh)
                ig = ap4.tile([128, 4, BP], F32, tag="ig")
                nc.vector.tensor_mul(ig, s_if[:, 0:4, :], tg)
                cnew = ap4.tile([128, 4, BP], F32, tag="c")
                nc.vector.tensor_add(cnew, cf, ig)
                # ---- o-tile ----
                o_ps = po.tile([128, 4, BP], F32, tag="o")
                nc.tensor.matmul(o_ps, eye_sb, wz[:, 12:16, t, :],
                                 start=True, stop=False, skip_group_check=True)
                for jo in range(4):
                    m = 12 + jo
                    for ko in range(4):
                        nc.tensor.matmul(
                            o_ps[:, jo, :],
                            u_sb[:, ko, 128 * m:128 * (m + 1)],
                            hprev[ko],
                            start=False, stop=(jo == 3 and ko == 3),
                            skip_group_check=True)
                so = ap4.tile([128, 4, BP], F32, tag="so")
                nc.scalar.activation(so, o_ps, AF.Sigmoid)
                th = ap4.tile([128, 4, BP], F32, tag="th")
                nc.scalar.activation(th, cnew, AF.Tanh)
                nc.vector.tensor_mul(hring[:, :, sl, :], so, th)
                cprev = cnew

                if t % 16 == 15 and KSTOP >= 3:
                    # stage h to DRAM for the phase-3 epilogue
                    t0 = t - 15
                    sl0 = 1 + (t0 % 32)
                    for ko in range(4):
                        nc.sync.dma_start(
                            out=bass.AP(tensor=hs.tensor,
                                        offset=hs.offset
                                        + (ko * 128 * T + t0) * BP,
                                        ap=[[T * BP, 128], [BP, 16], [1, BP]]),
                            in_=hring[:, ko, sl0:sl0 + 16, :])

            # ---------- phase 3 epilogue ----------
            if KSTOP >= 3:
                for blk in range(T // 16):
                    t0 = 16 * blk
                    hb = rd.tile([128, 4, 16, BP], F16, tag="hb")
                    for ko in range(4):
                        nc.sync.dma_start(
                            out=hb[:, ko, :, :],
                            in_=bass.AP(tensor=hs.tensor,
                                        offset=hs.offset
                                        + (ko * 128 * T + t0) * BP,
                                        ap=[[T * BP, 128], [BP, 16], [1, BP]]))
                    s_ps = sps.tile([128, S], F32, tag="sps")
                    for ko in range(4):
                        nc.tensor.matmul(
                            s_ps, hb[:, ko, :, :], wo_sb[:, ko, :],
                            start=(ko == 0), stop=False)
                    nc.tensor.matmul(s_ps, ones1, bout16,
                                     start=False, stop=True)
                    sig = rd.tile([128, S], F32, tag="sig")
                    nc.scalar.activation(sig, s_ps, AF.Sigmoid)
                    q_t = rd.tile([128, S], F32, tag="qt")
                    nc.sync.dma_start(
                        out=q_t,
                        in_=bass.AP(tensor=q.tensor,
                                    offset=q.offset + t0 * S,
                                    ap=[[S, 16], [T * S, BP], [1, S]]),
                    )
                    prod = rd.tile([128, S], F32, tag="prod")
                    ycol = rd.tile([128, 1], F32, tag="ycol")
                    nc.vector.tensor_mul(prod, sig, q_t)
                    nc.vector.tensor_reduce(ycol, prod, mybir.AxisListType.X,
                                            ALU.add)
                    nc.sync.dma_start(out=y[t0:t0 + 16, :], in_=ycol)


_CACHE = {}


def _get_nc(T=T_FULL):
    if T not in _CACHE:
        _CACHE[T] = _build(T)
    return _CACHE[T]


def kernel(x, delta, q, W_emb, b_emb, W_lstm, U_lstm, b_lstm, W_out, b_out):
    T = x.shape[1]
    nc = _get_nc(T)
    shared = dict(
        W_emb=np.ascontiguousarray(W_emb, np.float32),
        b_emb=np.ascontiguousarray(b_emb, np.float32),
        W_lstm=np.ascontiguousarray(W_lstm, np.float32),
        U_lstm=np.ascontiguousarray(U_lstm, np.float32),
        b_lstm=np.ascontiguousarray(b_lstm, np.float32),
        W_out=np.ascontiguousarray(W_out, np.float32),
        b_out=np.ascontiguousarray(b_out, np.float32),
    )
    in_maps = []
    for c in range(N_CORES):
        sl = slice(BP * c, BP * (c + 1))
        in_maps.append(dict(
            x=np.ascontiguousarray(x[sl], np.float32),
            delta=np.ascontiguousarray(np.asarray(delta)[sl, :, 0], np.float32),
            q=np.ascontiguousarray(q[sl], np.float32),
            **shared,
        ))
    res = run_bass_kernel_spmd(nc, in_maps, core_ids=list(range(N_CORES)))
    out = np.empty((x.shape[0], T, 1), np.float32)
    for c in range(N_CORES):
        out[BP * c:BP * (c + 1), :, 0] = res.results[c]["y"].T
    return out


# revision 26
# speedup vs baseline: 1.0670x; 1.0067x over previous
"""DKT (Deep Knowledge Tracing) accumulate-concat model on 8 Trainium2 cores.

Model (per example): one-hot interactions x[t] (2S=1024), query one-hots q,
  emb   = x @ W_emb + b_emb
  count = cumulative count state (c_t = sum(x_t)*c_{t-1} + x_t; x one-hot => cumsum)
  z     = [emb, log1p(count), log1p(delta)]
  h     = LSTM(z)                      (Keras gates i,f,g,o; unit forget bias)
  y     = sum(sigmoid(h @ W_out + b_out) * q, -1)

Sharding: data-parallel over batch. 8 cores x 8 examples. Weights replicated.

Device algorithm per core (B'=8 examples):
  Phase 0: cast weights to fp16; W_lstm re-tiled into DRAM as
    [16 m][128 p][10 kc][128 c] so phase 1 reads one contiguous-per-partition
    320KB DMA per m-group (2.5KB bursts).
  Phase 1 (per example):
    countT[2S, T] = x^T cumsum over t as ONE matmul (lhsT = x fp8 one-hot,
      rhs = fp8 upper-triangular ones; exact, fp32 psum).
    zt[2+mc] = log1p(countT) (ACT Ln, bias=1); cnt16 = fp16 copy of countT.
    embT = column-diff(W_emb^T @ cnt16) + b_emb   (diff recovers one-hot xT)
    WzT[2048, T] = W_lstm^T z + bias, fp16 in SBUF wz[p, m, t, b].
  Phase 2 (recurrence): per step, three PSUM tiles (i+f shared, g, o).
    Each tile's accumulation group starts with an identity matmul that loads
    wz[t] into PSUM, then U^T h matmuls accumulate on top. ACT reads PSUM
    directly: one sigmoid over [i|f], tanh(g), tanh(c), sigmoid(o) -- 4 ACT
    ops/step, no DVE psum reads. DVE: cf, ig, cnew, h (SBUF-only, ~60ns).
  Phase 3 (fused, every 16 steps): s = h @ W_out + b_out,
    y = reduce_sum(sigmoid(s) * q); contiguous [16, 8] store.

Output DRAM tensor is [T, 8] (t-major); host transposes.
"""

import os
import sys

sys.path.insert(0, "/opt/trn_rl_repo")

KSTOP = int(os.environ.get("KSTOP", "9"))  # debug: stop after phase N

import numpy as np

import concourse.bass as bass
import concourse.tile as tile
from concourse import bacc, mybir
from concourse.bass_utils import run_bass_kernel_spmd

F32 = mybir.dt.float32
F16 = mybir.dt.float16
F8 = mybir.dt.float8e4
AF = mybir.ActivationFunctionType
ALU = mybir.AluOpType

N_CORES = 8
B_FULL, T_FULL, S = 64, 512, 512
S2 = 2 * S          # 1024 one-hot dim
DE = 256            # emb dim
H = 512             # lstm hidden
G4 = 4 * H          # 2048 gate cols
BP = 8              # examples per core


def _build(T=T_FULL):
    KT = T // 128           # K-tiles over time for count matmul
    nc = bacc.Bacc("TRN2", target_bir_lowering=False, debug=False)

    x_h = nc.dram_tensor("x", [BP, T, S2], F32, kind="ExternalInput")
    d_h = nc.dram_tensor("delta", [BP, T], F32, kind="ExternalInput")
    q_h = nc.dram_tensor("q", [BP, T, S], F32, kind="ExternalInput")
    we_h = nc.dram_tensor("W_emb", [S2, DE], F32, kind="ExternalInput")
    be_h = nc.dram_tensor("b_emb", [DE], F32, kind="ExternalInput")
    wl_h = nc.dram_tensor("W_lstm", [S2 + DE + 1, G4], F32, kind="ExternalInput")
    ul_h = nc.dram_tensor("U_lstm", [H, G4], F32, kind="ExternalInput")
    bl_h = nc.dram_tensor("b_lstm", [G4], F32, kind="ExternalInput")
    wo_h = nc.dram_tensor("W_out", [H, S], F32, kind="ExternalInput")
    bo_h = nc.dram_tensor("b_out", [S], F32, kind="ExternalInput")
    y_h = nc.dram_tensor("y", [T, BP], F32, kind="ExternalOutput")

    # fp16 weight scratch in DRAM; W_lstm tiled [16 m][128 p][10 kc][128 c]
    wlt_h = nc.dram_tensor("wl_t16", [16, 128, 10, 128], F16, kind="Internal")
    ulh_h = nc.dram_tensor("ul_f16", [H, G4], F16, kind="Internal")
    # staged h for the phase-3 epilogue: [4 ko][128 p][T][BP]
    hs_h = nc.dram_tensor("h_stage", [4, 128, T, BP], F16, kind="Internal")

    import ml_dtypes
    tri_h = nc.inline_tensor(
        np.triu(np.ones((T, T))).astype(ml_dtypes.float8_e4m3), name="triu")
    eye_h = nc.inline_tensor(np.eye(128, dtype=np.float16), name="eye128")

    x, d, q = x_h.ap(), d_h.ap(), q_h.ap()
    we, be, wl, ul, bl = we_h.ap(), be_h.ap(), wl_h.ap(), ul_h.ap(), bl_h.ap()
    wo, bo, y = wo_h.ap(), bo_h.ap(), y_h.ap()
    wlt, ulh, tri, eye = wlt_h.ap(), ulh_h.ap(), tri_h.ap(), eye_h.ap()
    hs = hs_h.ap()

    with tile.TileContext(nc) as tc:
        _kernel_body(nc, tc, T, KT, x, d, q, we, be, wl, ul, bl, wo, bo, y,
                     wlt, ulh, tri, eye, hs)
    nc.compile()
    return nc


def _kernel_body(nc, tc, T, KT, x, d, q, we, be, wl, ul, bl, wo, bo, y,
                 wlt, ulh, tri, eye, hs):
    from contextlib import ExitStack

    ctx = ExitStack()
    with ctx:
        # ---------- persistent pools ----------
        per = ctx.enter_context(tc.tile_pool(name="persist", bufs=1))
        wz_pool = ctx.enter_context(tc.tile_pool(name="wz", bufs=1))

        wz = wz_pool.tile([128, 16, T, BP], F16)          # 128KB/part at T=512
        wo_sb = per.tile([128, 4, S], F16)                # 4KB/part
        wemb = per.tile([128, 8, DE], F16)                # 4KB/part
        tri_sb = per.tile([128, KT, T], F8)               # 2KB/part at T=512
        eye_sb = per.tile([128, 128], F16)
        wd = per.tile([1, G4], F16)
        bembT = per.tile([128, 2], F32)
        bembT16 = per.tile([128, 2], F16)
        blstm = per.tile([128, 16], F32)
        bias_g = per.tile([128, 16], F32)                 # b_lstm + b_emb@W1
        bout16 = per.tile([1, S], F16)
        ones1 = per.tile([1, 128], F16)
        nc.vector.memset(ones1, 1.0)
        nc.sync.dma_start(out=eye_sb, in_=eye)

        # ---------- phase 0: load + cast weights ----------
        with tc.tile_pool(name="ph0", bufs=1) as p0, \
             tc.tile_pool(name="ph0ps", bufs=1, space="PSUM") as p0ps:
            # U -> fp16 DRAM scratch (loaded to SBUF at phase 2 start)
            for ko in range(4):
                t32 = p0.tile([128, G4], F32, tag="w32")
                t16 = p0.tile([128, G4], F16, tag="w16")
                nc.sync.dma_start(out=t32, in_=ul[128 * ko:128 * (ko + 1), :])
                nc.scalar.activation(t16, t32, AF.Copy)
                nc.sync.dma_start(out=ulh[128 * ko:128 * (ko + 1), :], in_=t16)
            for ko in range(4):
                t32 = p0.tile([128, S], F32, tag="wo32")
                nc.sync.dma_start(out=t32, in_=wo[128 * ko:128 * (ko + 1), :])
                nc.scalar.activation(wo_sb[:, ko, :], t32, AF.Copy)
            for mc in range(8):
                t32 = p0.tile([128, DE], F32, tag="we32")
                nc.sync.dma_start(out=t32, in_=we[128 * mc:128 * (mc + 1), :])
                nc.scalar.activation(wemb[:, mc, :], t32, AF.Copy)
            for kt in range(KT):
                nc.sync.dma_start(out=tri_sb[:, kt, :],
                                  in_=tri[128 * kt:128 * (kt + 1), :])
            # W_lstm rows 0..1279 -> tiled fp16 DRAM scratch:
            # element (m, p, kc, c) at m*163840 + p*1280 + kc*128 + c
            for kc in range(10):
                t32 = p0.tile([128, G4], F32, tag="w32")
                t16 = p0.tile([128, G4], F16, tag="w16")
                nc.sync.dma_start(out=t32, in_=wl[128 * kc:128 * (kc + 1), :])
                nc.scalar.activation(t16, t32, AF.Copy)
                nc.sync.dma_start(
                    out=bass.AP(tensor=wlt.tensor,
                                offset=wlt.offset + kc * 128,
                                ap=[[1280, 128], [163840, 16], [1, 128]]),
                    in_=t16)
            # delta row of W_lstm
            t32 = p0.tile([1, G4], F32, tag="wd32")
            nc.sync.dma_start(out=t32, in_=wl[1280:1281, :])
            nc.scalar.activation(wd, t32, AF.Copy)
            # biases
            t32 = p0.tile([128, 2], F32, tag="be32")
            nc.sync.dma_start(
                out=t32,
                in_=bass.AP(tensor=be.tensor, offset=be.offset,
                            ap=[[1, 128], [128, 2]]),
            )
            nc.vector.tensor_copy(bembT, t32)
            nc.vector.tensor_copy(bembT16, t32)
            nc.sync.dma_start(
                out=blstm,
                in_=bass.AP(tensor=bl.tensor, offset=bl.offset,
                            ap=[[1, 128], [128, 16]]),
            )
            t32 = p0.tile([1, S], F32, tag="bo32")
            nc.sync.dma_start(out=t32, in_=bo[None, :])
            nc.scalar.activation(bout16, t32, AF.Copy)
            # bias_g[:, m] = b_lstm[m] + (b_emb @ W1)[m]
            for m in range(16):
                bias_ps = p0ps.tile([128, 1], F32, tag="biasps")
                for kc in range(2):
                    wt16 = p0.tile([128, 128], F16, tag="wbt")
                    nc.sync.dma_start(
                        out=wt16,
                        in_=bass.AP(tensor=wlt.tensor,
                                    offset=wlt.offset + m * 163840 + kc * 128,
                                    ap=[[1280, 128], [1, 128]]))
                    nc.tensor.matmul(bias_ps, wt16, bembT16[:, kc:kc + 1],
                                     start=(kc == 0), stop=(kc == 1))
                nc.vector.tensor_add(bias_g[:, m:m + 1], bias_ps,
                                     blstm[:, m:m + 1])

        if KSTOP < 1:
            return
        # ---------- phase 1: Wz precompute ----------
        with tc.tile_pool(name="p1sb", bufs=1) as p1, \
             tc.tile_pool(name="p1s", bufs=1) as p1s, \
             tc.tile_pool(name="p1x", bufs=2) as p1x, \
             tc.tile_pool(name="p1w", bufs=4) as p1w, \
             tc.tile_pool(name="p1cnt", bufs=1, space="PSUM") as pps, \
             tc.tile_pool(name="p1mm", bufs=3, space="PSUM") as pps2:
            zt = {}
            for kc in range(10):
                zt[kc] = p1.tile([128, T], F16, tag=f"zt{kc}", name=f"zt{kc}")
            for ex in range(BP):
                ld32 = p1x.tile([1, T], F32, tag="ld32")
                nc.sync.dma_start(out=ld32, in_=d[ex:ex + 1, :])
                ldt = p1x.tile([1, T], F16, tag="ldt")
                nc.scalar.activation(ldt, ld32, AF.Ln, bias=1.0)
                # -- load + cast x once (fp8: one-hot, exact) --
                x8 = p1s.tile([128, KT, S2], F8, tag="x8")
                for kt in range(KT):
                    x32 = p1x.tile([128, S2], F32, tag="x32")
                    nc.sync.dma_start(
                        out=x32, in_=x[ex, 128 * kt:128 * (kt + 1), :])
                    nc.scalar.activation(x8[:, kt, :], x32, AF.Copy)
                # -- count matmul + log1p + fp16 copy --
                cnt16 = p1s.tile([128, 8, T], F16, tag="cnt16")
                for mch in range(2):
                    cnt_ps = [pps.tile([128, T], F32, tag=f"cnt{i}",
                                       name=f"cnt{i}")
                              for i in range(4)]
                    for kt in range(KT):
                        for i in range(4):
                            mc = 4 * mch + i
                            nc.tensor.matmul(
                                cnt_ps[i], x8[:, kt, 128 * mc:128 * (mc + 1)],
                                tri_sb[:, kt, :],
                                start=(kt == 0), stop=(kt == KT - 1))
                    for i in range(4):
                        mc = 4 * mch + i
                        nc.scalar.activation(zt[2 + mc], cnt_ps[i], AF.Ln,
                                             bias=1.0)
                        nc.vector.tensor_copy(cnt16[:, mc, :], cnt_ps[i])
                # -- embT = diff(W_emb^T @ cnt16) + b_emb --
                for m2 in range(2):
                    e_ps = pps2.tile([128, T], F32, tag="mm")
                    for mc in range(8):
                        nc.tensor.matmul(
                            e_ps, wemb[:, mc, 128 * m2:128 * (m2 + 1)],
                            cnt16[:, mc, :],
                            start=(mc == 0), stop=(mc == 7))
                    e_sb = p1x.tile([128, T], F32, tag="esb")
                    nc.vector.tensor_copy(e_sb, e_ps)
                    nc.vector.tensor_scalar_add(zt[m2][:, 0:1], e_sb[:, 0:1],
                                                bembT[:, m2:m2 + 1])
                    nc.vector.tensor_sub(zt[m2][:, 1:T], e_sb[:, 1:T],
                                         e_sb[:, 0:T - 1])
                    nc.vector.tensor_scalar_add(zt[m2][:, 1:T], zt[m2][:, 1:T],
                                                bembT[:, m2:m2 + 1])
                # -- big matmul: WzT = W^T z + bias --
                for m in range(16):
                    wtile = p1w.tile([128, 10, 128], F16, tag="wlt")
                    nc.sync.dma_start(
                        out=wtile,
                        in_=bass.AP(tensor=wlt.tensor,
                                    offset=wlt.offset + m * 163840,
                                    ap=[[1280, 128], [128, 10], [1, 128]]))
                    b_ps = pps2.tile([128, T], F32, tag="mm")
                    for kc in range(10):
                        nc.tensor.matmul(b_ps, wtile[:, kc, :], zt[kc],
                                         start=(kc == 0), stop=False)
                    nc.tensor.matmul(b_ps, wd[:, 128 * m:128 * (m + 1)],
                                     ldt,
                                     start=False, stop=True)
                    nc.vector.tensor_scalar_add(wz[:, m, :, ex], b_ps,
                                                bias_g[:, m:m + 1])

        if KSTOP < 2:
            return
        # ---------- phase 2 + 3: recurrence with fused output ----------
        with tc.tile_pool(name="rec", bufs=1) as rp, \
             tc.tile_pool(name="recd", bufs=3) as rd, \
             tc.tile_pool(name="act4", bufs=3) as ap4, \
             tc.tile_pool(name="pif", bufs=2, space="PSUM") as pif, \
             tc.tile_pool(name="pg", bufs=2, space="PSUM") as pg, \
             tc.tile_pool(name="po", bufs=2, space="PSUM") as po, \
             tc.tile_pool(name="sps", bufs=2, space="PSUM") as sps:
            # U fp16 from DRAM scratch (SBUF space freed by phase-1 pools)
            u_sb = rp.tile([128, 4, G4], F16)             # 16KB/part
            for ko in range(4):
                nc.sync.dma_start(out=u_sb[:, ko, :],
                                  in_=ulh[128 * ko:128 * (ko + 1), :])
            hring = rp.tile([128, 4, 33, BP], F16)
            c0 = rp.tile([128, 4, BP], F32)
            nc.vector.memset(hring[:, :, 0, :], 0.0)
            nc.vector.memset(c0, 0.0)
            cprev = c0

            for t in range(T):
                sl_prev = 1 + ((t - 1) % 32) if t > 0 else 0
                sl = 1 + (t % 32)
                hprev = [hring[:, ko, sl_prev, :] for ko in range(4)]
                # ---- if-tile: identity loads wz, then U^T h accumulates ----
                if_ps = pif.tile([128, 8, BP], F32, tag="if")
                nc.tensor.matmul(if_ps, eye_sb, wz[:, 0:8, t, :],
                                 start=True, stop=False, skip_group_check=True)
                for jo in range(8):
                    for ko in range(4):
                        nc.tensor.matmul(
                            if_ps[:, jo, :],
                            u_sb[:, ko, 128 * jo:128 * (jo + 1)],
                            hprev[ko],
                            start=False, stop=(jo == 7 and ko == 3),
                            skip_group_check=True)
                s_if = ap4.tile([128, 8, BP], F32, tag="sif")
                nc.scalar.activation(s_if, if_ps, AF.Sigmoid)
                cf = ap4.tile([128, 4, BP], F32, tag="cf")
                nc.vector.tensor_mul(cf, s_if[:, 4:8, :], cprev)
                # ---- g-tile ----
                g_ps = pg.tile([128, 4, BP], F32, tag="g")
                nc.tensor.matmul(g_ps, eye_sb, wz[:, 8:12, t, :],
                                 start=True, stop=False, skip_group_check=True)
                for jo in range(4):
                    m = 8 + jo
                    for ko in range(4):
                        nc.tensor.matmul(
                            g_ps[:, jo, :],
                            u_sb[:, ko, 128 * m:128 * (m + 1)],
                            hprev[ko],
                            start=False, stop=(jo == 3 and ko == 3),
                            skip_group_check=True)
                tg = ap4.tile([128, 4, BP], F32, tag="tg")
                nc.scalar.activation(tg, g_ps, AF.Tanh)
                ig = ap4.tile([128, 4, BP], F32, tag="ig")
                nc.vector.tensor_mul(ig, s_if[:, 0:4, :], tg)
                cnew = ap4.tile([128, 4, BP], F32, tag="c")
                nc.vector.tensor_add(cnew, cf, ig)
                # ---- o-tile ----
                o_ps = po.tile([128, 4, BP], F32, tag="o")
                nc.tensor.matmul(o_ps, eye_sb, wz[:, 12:16, t, :],
                                 start=True, stop=False, skip_group_check=True)
                for jo in range(4):
                    m = 12 + jo
                    for ko in range(4):
                        nc.tensor.matmul(
                            o_ps[:, jo, :],
                            u_sb[:, ko, 128 * m:128 * (m + 1)],
                            hprev[ko],
                            start=False, stop=(jo == 3 and ko == 3),
                            skip_group_check=True)
                so = ap4.tile([128, 4, BP], F32, tag="so")
                nc.scalar.activation(so, o_ps, AF.Sigmoid)
                th = ap4.tile([128, 4, BP], F32, tag="th")
                nc.scalar.activation(th, cnew, AF.Tanh)
                nc.vector.tensor_mul(hring[:, :, sl, :], so, th)
                cprev = cnew

                if t % 16 == 15 and KSTOP >= 3:
                    # stage h to DRAM for the phase-3 epilogue
                    t0 = t - 15
                    sl0 = 1 + (t0 % 32)
                    for ko in range(4):
                        nc.sync.dma_start(
                            out=bass.AP(tensor=hs.tensor,
                                        offset=hs.offset
                                        + (ko * 128 * T + t0) * BP,
                                        ap=[[T * BP, 128], [BP, 16], [1, BP]]),
                            in_=hring[:, ko, sl0:sl0 + 16, :])

            # ---------- phase 3 epilogue ----------
            if KSTOP >= 3:
                for blk in range(T // 16):
                    t0 = 16 * blk
                    hb = rd.tile([128, 4, 16, BP], F16, tag="hb")
                    for ko in range(4):
                        nc.sync.dma_start(
                            out=hb[:, ko, :, :],
                            in_=bass.AP(tensor=hs.tensor,
                                        offset=hs.offset
                                        + (ko * 128 * T + t0) * BP,
                                        ap=[[T * BP, 128], [BP, 16], [1, BP]]))
                    s_ps = sps.tile([128, S], F32, tag="sps")
                    for ko in range(4):
                        nc.tensor.matmul(
                            s_ps, hb[:, ko, :, :], wo_sb[:, ko, :],
                            start=(ko == 0), stop=False)
                    nc.tensor.matmul(s_ps, ones1, bout16,
                                     start=False, stop=True)
                    sig = rd.tile([128, S], F32, tag="sig")
                    nc.scalar.activation(sig, s_ps, AF.Sigmoid)
                    q_t = rd.tile([128, S], F32, tag="qt")
                    nc.sync.dma_start(
                        out=q_t,
                        in_=bass.AP(tensor=q.tensor,
                                    offset=q.offset + t0 * S,
                                    ap=[[S, 16], [T * S, BP], [1, S]]),
                    )
                    prod = rd.tile([128, S], F32, tag="prod")
                    ycol = rd.tile([128, 1], F32, tag="ycol")
                    nc.vector.tensor_mul(prod, sig, q_t)
                    nc.vector.tensor_reduce(ycol, prod, mybir.AxisListType.X,
                                            ALU.add)
                    nc.sync.dma_start(out=y[t0:t0 + 16, :], in_=ycol)


_CACHE = {}


def _get_nc(T=T_FULL):
    if T not in _CACHE:
        _CACHE[T] = _build(T)
    return _CACHE[T]


def kernel(x, delta, q, W_emb, b_emb, W_lstm, U_lstm, b_lstm, W_out, b_out):
    T = x.shape[1]
    nc = _get_nc(T)
    shared = dict(
        W_emb=np.ascontiguousarray(W_emb, np.float32),
        b_emb=np.ascontiguousarray(b_emb, np.float32),
        W_lstm=np.ascontiguousarray(W_lstm, np.float32),
        U_lstm=np.ascontiguousarray(U_lstm, np.float32),
        b_lstm=np.ascontiguousarray(b_lstm, np.float32),
        W_out=np.ascontiguousarray(W_out, np.float32),
        b_out=np.ascontiguousarray(b_out, np.float32),
    )
    in_maps = []
    for c in range(N_CORES):
        sl = slice(BP * c, BP * (c + 1))
        in_maps.append(dict(
            x=np.ascontiguousarray(x[sl], np.float32),
            delta=np.ascontiguousarray(np.asarray(delta)[sl, :, 0], np.float32),
            q=np.ascontiguousarray(q[sl], np.float32),
            **shared,
        ))
    res = run_bass_kernel_spmd(nc, in_maps, core_ids=list(range(N_CORES)))
    out = np.empty((x.shape[0], T, 1), np.float32)
    for c in range(N_CORES):
        out[BP * c:BP * (c + 1), :, 0] = res.results[c]["y"].T
    return out
